# revision 17
# baseline (speedup 1.0000x reference)
"""Trainium2 Bass kernel: 12-head self-attention block (B=2, N=4096, C=768).

Sharding: token-parallel over the 8192 (batch, token) rows. Core c (0..7)
handles batch c//4, query rows [(c%4)*1024, (c%4+1)*1024). Every core
redundantly computes K/V for its WHOLE batch - zero collectives. The host
rotates each core's token order so its own 1024 query tokens come first
(attention is permutation-invariant over keys).

v3 (vs v2 629us): PE pair-concurrency + balanced ACT/DVE/GpSimd duties.
  - phase A: x streamed in 512-token half-quarters (double-buffered), Wqkv
    DMA'd in q/k/v column thirds (q first) so the PE ramps immediately.
  - phase C per (head-pair, query-half, key-chunk): three concurrent MM
    pairs - scores (row groups 0/64), AV (col groups 0/64), denominator
    (ones lhsT, M=64 -> the denominator lands replicated across 64
    partitions = broadcast for free). exp: ACT does head A batched
    [128,1024] per 2 chunks; DVE does head B via Schraudolph int16
    bit-trick. Division: reciprocal_approx_fast + one gpsimd multiply per
    pass, deferred off the critical path. No partition-shift DMA: head B
    lands on partitions 64-127 natively.
  - PSUM: scA 2x2 + scB 2x1 + av 1 + den 1 = 8 banks.
  - Wproj/bias prefetched before phase C.
"""

import sys

import numpy as np

try:
    import concourse  # noqa: F401
except ImportError:  # pragma: no cover
    sys.path.insert(0, "/opt/trn_rl_repo")

import concourse.bass as bass  # noqa: F401
import concourse.mybir as mybir
import concourse.tile as tile
from concourse import bacc
from concourse.bass_utils import run_bass_kernel_spmd

B, N, C = 2, 4096, 768
H, D = 12, 64
NT = 1024  # query tokens per core
SCALE = float(D) ** -0.5
NCORES = 8
KC = N // 128  # 32 key chunks per batch
VW = D + 1  # v_aug row width per head: [v(64), ones]

F32 = mybir.dt.float32
F32R = mybir.dt.float32r
BF16 = mybir.dt.bfloat16
FP16 = mybir.dt.float16
I16 = mybir.dt.int16
EXP = mybir.ActivationFunctionType.Exp
COPY = mybir.ActivationFunctionType.Copy
MUL = mybir.AluOpType.mult
ADD = mybir.AluOpType.add

# Schraudolph exp in bf16 bits: e_bits = int16(A_EXP * s + B_EXP)
A_EXP = 1.4426950408889634 * 128.0
B_EXP = 16256.0 - 6.0


def build_graph():
    nc = bacc.Bacc(
        "TRN2", target_bir_lowering=False, debug=False, num_devices=NCORES
    )

    xT_e = nc.declare_dram_parameter("xT", [C, N], FP16, isOutput=False)
    wqkv_e = nc.declare_dram_parameter("Wqkv", [C, 3 * C], FP16, isOutput=False)
    wproj_e = nc.declare_dram_parameter("Wproj", [C, C], F32R, isOutput=False)
    bproj_e = nc.declare_dram_parameter("bproj", [1, C], F32, isOutput=False)
    out_e = nc.declare_dram_parameter("out", [NT, C], F32, isOutput=True)

    with tile.TileContext(nc) as tc:
        with tc.tile_pool(name="persist", bufs=1) as persist:
            kT_sb = persist.tile([128, 6, N], FP16, tag="kT")  # 48K/part
            vt_sb = persist.tile([128, KC, H, VW], BF16, tag="vt")  # 48.75K
            qT_sb = persist.tile([128, 6, NT], FP16, tag="qT")  # 12K
            ones64 = persist.tile([128, 64], BF16, tag="ones64")
            nc.vector.memset(ones64[:], 1.0)
            nc.vector.memset(vt_sb[:, :, :, D : D + 1], 1.0)

            _phase_a(nc, tc, xT_e, wqkv_e, kT_sb, vt_sb, qT_sb)

            with (
                tc.tile_pool(name="tokp", bufs=1) as tokp,
                tc.tile_pool(name="pd_w", bufs=1) as pdw,
            ):
                tokT = tokp.tile([128, 6, NT], F32R, tag="tokT")  # 24K
                # prefetch Wproj/bias during phase C
                wproj_sb = pdw.tile([128, 6, C], F32R, tag="wproj")
                bproj_sb = pdw.tile([1, C], F32, tag="bproj")
                bproj_bc = pdw.tile([128, C], F32, tag="bproj_bc")
                nc.sync.dma_start(bproj_sb[:], bproj_e[:])
                nc.gpsimd.partition_broadcast(bproj_bc[:], bproj_sb[:])
                for cc in range(6):
                    nc.sync.dma_start(
                        wproj_sb[:, cc, :],
                        wproj_e[cc * 128 : (cc + 1) * 128, :],
                    )
                _phase_c(nc, tc, kT_sb, vt_sb, qT_sb, tokT, ones64)
                _phase_d(nc, tc, tokT, wproj_sb, bproj_bc, out_e)
    nc.finalize()
    return nc


def _phase_a(nc, tc, xT_e, wqkv_e, kT_sb, vt_sb, qT_sb):
    """qkv projection in fp16 (FWL-eligible weight loads), 1024-token
    quarters, double-buffered x, fat DMA slabs (>=2KB per partition)."""
    with (
        tc.tile_pool(name="pa_w", bufs=1) as paw,
        tc.tile_pool(name="pa_x", bufs=2) as pax,
        tc.tile_pool(name="pa_psk", bufs=2, space="PSUM") as papk,
        tc.tile_pool(name="pa_psv", bufs=2, space="PSUM") as papv,
    ):
        wqkv_sb = paw.tile([128, 6, 3 * C], FP16, tag="wqkv")  # 27.6K
        for kc in range(6):
            nc.sync.dma_start(
                wqkv_sb[:, kc, :], wqkv_e[kc * 128 : (kc + 1) * 128, :]
            )

        ncopy = [0]

        def copy_out(dst, src_ap):
            ncopy[0] += 1
            if ncopy[0] & 1:
                nc.vector.tensor_copy(dst, src_ap)
            else:
                nc.scalar.activation(dst, src_ap, COPY)

        for tq in range(4):  # 1024-token quarters
            t0 = tq * NT
            xq = pax.tile([128, 6, NT], FP16, tag="xq")  # 12K x2
            for kc in range(6):
                nc.sync.dma_start(
                    xq[:, kc, :], xT_e[kc * 128 : (kc + 1) * 128, t0 : t0 + NT]
                )

            # q (quarter 0 only, pre-scaled by SCALE) + k columns
            ccs = range(12) if tq == 0 else range(6, 12)
            for cc in ccs:
                pj = papk.tile([128, NT], F32, tag="pj")
                for kc in range(6):
                    for th in range(2):
                        nc.tensor.matmul(
                            pj[:, th * 512 : (th + 1) * 512],
                            wqkv_sb[:, kc, cc * 128 : (cc + 1) * 128],
                            xq[:, kc, th * 512 : (th + 1) * 512],
                            start=(kc == 0),
                            stop=(kc == 5),
                        )
                if cc < 6:
                    nc.scalar.activation(
                        qT_sb[:, cc, t0 : t0 + NT], pj[:], COPY, scale=SCALE
                    )
                else:
                    copy_out(kT_sb[:, cc - 6, t0 : t0 + NT], pj[:])

            # v (token-major), 8 chunks of 128 tokens
            for tcn in range(8):
                pj = papv.tile([128, C], F32, tag="pjv")
                for kc in range(6):
                    for c0, c1 in ((0, 512), (512, C)):
                        nc.tensor.matmul(
                            pj[:, c0:c1],
                            xq[:, kc, tcn * 128 : (tcn + 1) * 128],
                            wqkv_sb[:, kc, 2 * C + c0 : 2 * C + c1],
                            start=(kc == 0),
                            stop=(kc == 5),
                        )
                copy_out(
                    vt_sb[:, tq * 8 + tcn, :, 0:D],
                    pj[:].rearrange("p (h d) -> p h d", d=D),
                )


def _phase_c(nc, tc, kT_sb, vt_sb, qT_sb, tokT, ones64):
    """Attention. Two query-half passes per head pair. Per key chunk kc:
    scores pair (row groups 0/64 overlap in the array), then hybrid AV:
    head A via ones-column v_aug (M=65, denominator free in row 64);
    head B as M=64 AV at cols 0-63 plus a replicated-denominator MM
    (ones lhsT, cols 64-127) sharing one PSUM bank - 2048 stream cycles
    per chunk instead of 2560. exp: ACT head A [128,1024]/2kc, DVE head B
    Schraudolph [128,512]/kc. PSUM: scA 2x2 + scB 2x1 + avA 1 + avB 1 = 8."""
    with (
        tc.tile_pool(name="scA", bufs=2, space="PSUM") as scpA,
        tc.tile_pool(name="scB", bufs=2, space="PSUM") as scpB,
        tc.tile_pool(name="avpA", bufs=1, space="PSUM") as avpA,
        tc.tile_pool(name="avpB", bufs=1, space="PSUM") as avpB,
        tc.tile_pool(name="e0p", bufs=3) as e0p,
        tc.tile_pool(name="e1p", bufs=6) as e1p,
        tc.tile_pool(name="avsb", bufs=4) as avsbp,
        tc.tile_pool(name="recp", bufs=2) as recp,
        tc.tile_pool(name="small", bufs=2) as smp,
    ):
        deferred = []  # (hp, qh, hd, av_sb) division work, run later

        def emit_div(hp, qh, hd, av_sb):
            qsl = slice(qh * 512, (qh + 1) * 512)
            den = recp.tile([1, 512], F32, tag="den")
            nc.vector.tensor_copy(den[:], av_sb[64:65, :])
            rec = recp.tile([1, 512], F32, tag="rec")
            nc.vector.reciprocal_approx_fast(rec[:], den[:])
            bc = smp.tile([64, 512], F32, tag="bc")
            nc.gpsimd.partition_broadcast(bc[:], rec[:])
            if hd == 0:
                nc.gpsimd.tensor_tensor(
                    out=tokT[0:64, hp, qsl], in0=av_sb[0:64, :], in1=bc[:],
                    op=MUL,
                )
            else:
                tmp = smp.tile([64, 512], F32R, tag="tmp")
                nc.gpsimd.tensor_tensor(
                    out=tmp[:], in0=av_sb[0:64, :], in1=bc[:], op=MUL
                )
                # partition-shifting copy (base 0 -> 64) via DMA
                nc.sync.dma_start(tokT[64:128, hp, qsl], tmp[:])

        for hp in range(6):
            for qh in range(2):
                qsl = slice(qh * 512, (qh + 1) * 512)
                avA = avpA.tile([65, 512], F32, tag="avA", name=f"avA_{hp}_{qh}")
                avB = avpB.tile([128, 512], F32, tag="avB", name=f"avB_{hp}_{qh}")

                def do_avdn(kc, e0s, e1s, avA=avA, avB=avB, hp=hp):
                    st, sp = kc == 0, kc == KC - 1
                    # order: avB (cols 0-63), den_B (cols 64-127), avA -
                    # each LDW overlaps the previous MM's stream
                    nc.tensor.matmul(
                        avB[0:64, :], vt_sb[:, kc, 2 * hp + 1, 0:D], e1s,
                        start=st, stop=sp, tile_position=(0, 0),
                    )
                    nc.tensor.matmul(
                        avB[64:128, :], ones64[:], e1s,
                        start=st, stop=sp, tile_position=(0, 64),
                    )
                    nc.tensor.matmul(
                        avA[:], vt_sb[:, kc, 2 * hp, :], e0s,
                        start=st, stop=sp,
                    )

                pend = []
                scA = None
                e1prev = None
                for kc in range(KC):
                    if kc in (8, 20) and deferred:
                        emit_div(*deferred.pop(0))
                    if pend and kc >= 2:
                        do_avdn(*pend.pop(0))
                    ksl = slice(kc * 128, (kc + 1) * 128)
                    half = (kc % 2) * 512
                    if kc % 2 == 0:
                        scA = scpA.tile([128, 1024], F32, tag="scA")
                    scB = scpB.tile([128, 512], F32, tag="scB")
                    nc.tensor.matmul(
                        scA[:, half : half + 512],
                        kT_sb[0:64, hp, ksl],
                        qT_sb[0:64, hp, qsl],
                        start=True,
                        stop=True,
                    )
                    nc.tensor.matmul(
                        scB[:],
                        kT_sb[64:128, hp, ksl],
                        qT_sb[64:128, hp, qsl],
                        start=True,
                        stop=True,
                    )
                    e1i = e1p.tile([128, 512], I16, tag="e1")
                    nc.vector.tensor_scalar(
                        e1i[:], scB[:], A_EXP, B_EXP, MUL, ADD
                    )
                    if kc % 2 == 1:
                        e0 = e0p.tile([128, 1024], BF16, tag="e0")
                        nc.scalar.activation(e0[:], scA[:], EXP, scale=1.0)
                        pend.append(
                            (kc - 1, e0[:, 0:512], e1prev[:].bitcast(BF16))
                        )
                        pend.append(
                            (kc, e0[:, 512:1024], e1i[:].bitcast(BF16))
                        )
                    e1prev = e1i
                while pend:
                    do_avdn(*pend.pop(0))

                # evacuate av PSUM promptly (rows 0-64: values + denominator)
                av_sbA = avsbp.tile([65, 512], F32, tag="av_sbA")
                av_sbB = avsbp.tile([65, 512], F32, tag="av_sbB")
                nc.scalar.activation(av_sbA[:], avA[:], COPY)
                nc.scalar.activation(av_sbB[:], avB[0:65, :], COPY)
                deferred.append((hp, qh, 0, av_sbA))
                deferred.append((hp, qh, 1, av_sbB))

        while deferred:
            emit_div(*deferred.pop(0))


def _phase_d(nc, tc, tokT, wproj_sb, bproj_bc, out_e):
    """out[t, c] = tokT.T @ Wproj + bproj."""
    with (
        tc.tile_pool(name="pd_psum", bufs=4, space="PSUM") as pdp,
        tc.tile_pool(name="pd_sbuf", bufs=4) as pds,
    ):
        for tcn in range(8):
            pj = pdp.tile([128, C], F32, tag="pd")
            for cc in range(6):
                for c0, c1 in ((0, 512), (512, C)):
                    nc.tensor.matmul(
                        pj[:, c0:c1],
                        tokT[:, cc, tcn * 128 : (tcn + 1) * 128],
                        wproj_sb[:, cc, c0:c1],
                        start=(cc == 0),
                        stop=(cc == 5),
                    )
            ot = pds.tile([128, C], F32, tag="ot")
            nc.vector.tensor_tensor(
                out=ot[:], in0=pj[:], in1=bproj_bc[:], op=ADD
            )
            nc.sync.dma_start(out_e[tcn * 128 : (tcn + 1) * 128, :], ot[:])


_CACHE = {}


def _get_graph():
    if "nc" not in _CACHE:
        _CACHE["nc"] = build_graph()
    return _CACHE["nc"]


def make_in_maps(x, W_qkv, W_proj, b_proj):
    x = np.asarray(x, dtype=np.float32)
    W_qkv = np.ascontiguousarray(np.asarray(W_qkv, dtype=np.float16))
    W_proj = np.ascontiguousarray(np.asarray(W_proj, dtype=np.float32))
    b_proj = np.asarray(b_proj, dtype=np.float32).reshape(1, C)
    in_maps = []
    for c in range(NCORES):
        bb, r0 = c // 4, (c % 4) * NT
        idx = np.r_[r0 : r0 + NT, 0:r0, r0 + NT : N]
        xT = np.ascontiguousarray(x[bb][idx].T.astype(np.float16))
        in_maps.append(
            {
                "xT": xT,
                "Wqkv": W_qkv,
                "Wproj": W_proj,
                "bproj": b_proj,
            }
        )
    return in_maps


def run(x, W_qkv, W_proj, b_proj, trace=False):
    nc = _get_graph()
    in_maps = make_in_maps(x, W_qkv, W_proj, b_proj)
    res = run_bass_kernel_spmd(
        nc, in_maps, core_ids=list(range(NCORES)), trace=trace
    )
    out = np.zeros((B, N, C), dtype=np.float32)
    for c in range(NCORES):
        bb, r0 = c // 4, (c % 4) * NT
        out[bb, r0 : r0 + NT, :] = res.results[c]["out"]
    return out, res


def kernel(x, W_qkv, W_proj, b_proj):
    out, _ = run(x, W_qkv, W_proj, b_proj, trace=False)
    return out


# revision 18
# speedup vs baseline: 1.3435x; 1.3435x over previous
"""Trainium2 Bass kernel: 12-head self-attention block (B=2, N=4096, C=768).

Sharding: token-parallel over the 8192 (batch, token) rows. Core c (0..7)
handles batch c//4, query rows [(c%4)*1024, (c%4+1)*1024). Every core
redundantly computes K/V for its WHOLE batch - zero collectives. The host
rotates each core's token order so its own 1024 query tokens come first
(attention is permutation-invariant over keys).

v3 (vs v2 629us): PE pair-concurrency + balanced ACT/DVE/GpSimd duties.
  - phase A: x streamed in 512-token half-quarters (double-buffered), Wqkv
    DMA'd in q/k/v column thirds (q first) so the PE ramps immediately.
  - phase C per (head-pair, query-half, key-chunk): three concurrent MM
    pairs - scores (row groups 0/64), AV (col groups 0/64), denominator
    (ones lhsT, M=64 -> the denominator lands replicated across 64
    partitions = broadcast for free). exp: ACT does head A batched
    [128,1024] per 2 chunks; DVE does head B via Schraudolph int16
    bit-trick. Division: reciprocal_approx_fast + one gpsimd multiply per
    pass, deferred off the critical path. No partition-shift DMA: head B
    lands on partitions 64-127 natively.
  - PSUM: scA 2x2 + scB 2x1 + av 1 + den 1 = 8 banks.
  - Wproj/bias prefetched before phase C.
"""

import sys

import numpy as np

try:
    import concourse  # noqa: F401
except ImportError:  # pragma: no cover
    sys.path.insert(0, "/opt/trn_rl_repo")

import concourse.bass as bass  # noqa: F401
import concourse.mybir as mybir
import concourse.tile as tile
from concourse import bacc
from concourse.bass_utils import run_bass_kernel_spmd

B, N, C = 2, 4096, 768
H, D = 12, 64
NT = 1024  # query tokens per core
SCALE = float(D) ** -0.5
NCORES = 8
KC = N // 128  # 32 key chunks per batch

F32 = mybir.dt.float32
F32R = mybir.dt.float32r
BF16 = mybir.dt.bfloat16
FP16 = mybir.dt.float16
I16 = mybir.dt.int16
EXP = mybir.ActivationFunctionType.Exp
COPY = mybir.ActivationFunctionType.Copy
MUL = mybir.AluOpType.mult
ADD = mybir.AluOpType.add

# Schraudolph exp in bf16 bits: e_bits = int16(A_EXP * s + B_EXP)
A_EXP = 1.4426950408889634 * 128.0
B_EXP = 16256.0 - 6.0


def build_graph():
    nc = bacc.Bacc(
        "TRN2", target_bir_lowering=False, debug=False, num_devices=NCORES
    )

    xT_e = nc.declare_dram_parameter("xT", [C, N], FP16, isOutput=False)
    wqkv_e = nc.declare_dram_parameter("Wqkv", [C, 3 * C], FP16, isOutput=False)
    wproj_e = nc.declare_dram_parameter("Wproj", [C, C], F32R, isOutput=False)
    bproj_e = nc.declare_dram_parameter("bproj", [1, C], F32, isOutput=False)
    out_e = nc.declare_dram_parameter("out", [NT, C], F32, isOutput=True)

    with tile.TileContext(nc) as tc:
        with tc.tile_pool(name="persist", bufs=1) as persist:
            kT_sb = persist.tile([128, 6, N], FP16, tag="kT")  # 48K/part
            vt_sb = persist.tile([128, KC, H, D], BF16, tag="vt")  # 48K
            qT_sb = persist.tile([128, 6, NT], FP16, tag="qT")  # 12K
            ones64 = persist.tile([128, 64], BF16, tag="ones64")
            nc.vector.memset(ones64[:], 1.0)

            _phase_a(nc, tc, xT_e, wqkv_e, kT_sb, vt_sb, qT_sb)

            with (
                tc.tile_pool(name="tokp", bufs=1) as tokp,
                tc.tile_pool(name="pd_w", bufs=1) as pdw,
            ):
                tokT = tokp.tile([128, 6, NT], F32R, tag="tokT")  # 24K
                # prefetch Wproj/bias during phase C
                wproj_sb = pdw.tile([128, 6, C], F32R, tag="wproj")
                bproj_sb = pdw.tile([1, C], F32, tag="bproj")
                bproj_bc = pdw.tile([128, C], F32, tag="bproj_bc")
                nc.sync.dma_start(bproj_sb[:], bproj_e[:])
                nc.gpsimd.partition_broadcast(bproj_bc[:], bproj_sb[:])
                for cc in range(6):
                    nc.sync.dma_start(
                        wproj_sb[:, cc, :],
                        wproj_e[cc * 128 : (cc + 1) * 128, :],
                    )
                _phase_c(nc, tc, kT_sb, vt_sb, qT_sb, tokT, ones64)
                _phase_d(nc, tc, tokT, wproj_sb, bproj_bc, out_e)
    nc.finalize()
    return nc


def _phase_a(nc, tc, xT_e, wqkv_e, kT_sb, vt_sb, qT_sb):
    """qkv projection in fp16 (FWL-eligible weight loads), 1024-token
    quarters, double-buffered x, fat DMA slabs (>=2KB per partition)."""
    with (
        tc.tile_pool(name="pa_w", bufs=1) as paw,
        tc.tile_pool(name="pa_x", bufs=2) as pax,
        tc.tile_pool(name="pa_psk", bufs=2, space="PSUM") as papk,
        tc.tile_pool(name="pa_psv", bufs=2, space="PSUM") as papv,
    ):
        wqkv_sb = paw.tile([128, 6, 3 * C], FP16, tag="wqkv")  # 27.6K
        for kc in range(6):
            nc.sync.dma_start(
                wqkv_sb[:, kc, :], wqkv_e[kc * 128 : (kc + 1) * 128, :]
            )

        ncopy = [0]

        def copy_out(dst, src_ap):
            ncopy[0] += 1
            if ncopy[0] & 1:
                nc.vector.tensor_copy(dst, src_ap)
            else:
                nc.scalar.activation(dst, src_ap, COPY)

        for tq in range(4):  # 1024-token quarters
            t0 = tq * NT
            xq = pax.tile([128, 6, NT], FP16, tag="xq")  # 12K x2
            for kc in range(6):
                nc.sync.dma_start(
                    xq[:, kc, :], xT_e[kc * 128 : (kc + 1) * 128, t0 : t0 + NT]
                )

            # q (quarter 0 only, pre-scaled by SCALE) + k columns
            ccs = range(12) if tq == 0 else range(6, 12)
            for cc in ccs:
                pj = papk.tile([128, NT], F32, tag="pj")
                for kc in range(6):
                    for th in range(2):
                        nc.tensor.matmul(
                            pj[:, th * 512 : (th + 1) * 512],
                            wqkv_sb[:, kc, cc * 128 : (cc + 1) * 128],
                            xq[:, kc, th * 512 : (th + 1) * 512],
                            start=(kc == 0),
                            stop=(kc == 5),
                        )
                if cc < 6:
                    nc.scalar.activation(
                        qT_sb[:, cc, t0 : t0 + NT], pj[:], COPY, scale=SCALE
                    )
                else:
                    copy_out(kT_sb[:, cc - 6, t0 : t0 + NT], pj[:])

            # v (token-major), 8 chunks of 128 tokens
            for tcn in range(8):
                pj = papv.tile([128, C], F32, tag="pjv")
                for kc in range(6):
                    for c0, c1 in ((0, 512), (512, C)):
                        nc.tensor.matmul(
                            pj[:, c0:c1],
                            xq[:, kc, tcn * 128 : (tcn + 1) * 128],
                            wqkv_sb[:, kc, 2 * C + c0 : 2 * C + c1],
                            start=(kc == 0),
                            stop=(kc == 5),
                        )
                copy_out(
                    vt_sb[:, tq * 8 + tcn, :, :],
                    pj[:].rearrange("p (h d) -> p h d", d=D),
                )


def _phase_c(nc, tc, kT_sb, vt_sb, qT_sb, tokT, ones64):
    """Attention. Two query-half passes per head pair. Per key chunk kc:
    scores pair (row groups 0/64), AV pair + denominator pair (col groups
    0/64; ones lhsT M=64 replicates the denominator = free broadcast).
    exp: ACT head A [128,1024]/2kc, DVE head B Schraudolph [128,512]/kc."""
    with (
        tc.tile_pool(name="scA", bufs=2, space="PSUM") as scpA,
        tc.tile_pool(name="scB", bufs=2, space="PSUM") as scpB,
        tc.tile_pool(name="avp", bufs=1, space="PSUM") as avp,
        tc.tile_pool(name="dnp", bufs=1, space="PSUM") as dnp,
        tc.tile_pool(name="e0p", bufs=3) as e0p,
        tc.tile_pool(name="e1p", bufs=6) as e1p,
        tc.tile_pool(name="avsb", bufs=2) as avsbp,
        tc.tile_pool(name="recp", bufs=2) as recp,
    ):
        deferred = []  # (hp, qh, av_sb, den_sb)

        def emit_div(hp, qh, av_sb, den_sb):
            qsl = slice(qh * 512, (qh + 1) * 512)
            rec = recp.tile([128, 512], F32, tag="rec")
            nc.vector.reciprocal_approx_fast(rec[:], den_sb[:])
            nc.gpsimd.tensor_tensor(
                out=tokT[:, hp, qsl], in0=av_sb[:], in1=rec[:], op=MUL
            )

        for hp in range(6):
            for qh in range(2):
                qsl = slice(qh * 512, (qh + 1) * 512)
                av = avp.tile([128, 512], F32, tag="av", name=f"av_{hp}_{qh}")
                dn = dnp.tile([128, 512], F32, tag="dn", name=f"dn_{hp}_{qh}")

                def do_avdn(kc, e0s, e1s, av=av, dn=dn, hp=hp):
                    st, sp = kc == 0, kc == KC - 1
                    nc.tensor.matmul(
                        av[0:64, :], vt_sb[:, kc, 2 * hp, :], e0s,
                        start=st, stop=sp, tile_position=(0, 0),
                    )
                    nc.tensor.matmul(
                        av[64:128, :], vt_sb[:, kc, 2 * hp + 1, :], e1s,
                        start=st, stop=sp, tile_position=(0, 64),
                    )
                    nc.tensor.matmul(
                        dn[0:64, :], ones64[:], e0s,
                        start=st, stop=sp, tile_position=(0, 0),
                    )
                    nc.tensor.matmul(
                        dn[64:128, :], ones64[:], e1s,
                        start=st, stop=sp, tile_position=(0, 64),
                    )

                pend = []
                scA = None
                for kc in range(KC):
                    if kc == 8 and deferred:
                        emit_div(*deferred.pop(0))
                    if pend and kc >= 2:
                        do_avdn(*pend.pop(0))
                    ksl = slice(kc * 128, (kc + 1) * 128)
                    half = (kc % 2) * 512
                    if kc % 2 == 0:
                        scA = scpA.tile([128, 1024], F32, tag="scA")
                    scB = scpB.tile([128, 512], F32, tag="scB")
                    nc.tensor.matmul(
                        scA[:, half : half + 512],
                        kT_sb[0:64, hp, ksl],
                        qT_sb[0:64, hp, qsl],
                        start=True,
                        stop=True,
                    )
                    nc.tensor.matmul(
                        scB[:],
                        kT_sb[64:128, hp, ksl],
                        qT_sb[64:128, hp, qsl],
                        start=True,
                        stop=True,
                    )
                    e1i = e1p.tile([128, 512], I16, tag="e1")
                    nc.vector.tensor_scalar(
                        e1i[:], scB[:], A_EXP, B_EXP, MUL, ADD
                    )
                    if kc % 2 == 1:
                        e0 = e0p.tile([128, 1024], BF16, tag="e0")
                        nc.scalar.activation(e0[:], scA[:], EXP, scale=1.0)
                        pend.append((kc - 1, e0[:, 0:512], e1prev[:].bitcast(BF16)))
                        pend.append((kc, e0[:, 512:1024], e1i[:].bitcast(BF16)))
                    e1prev = e1i
                while pend:
                    do_avdn(*pend.pop(0))

                # evacuate av/den PSUM promptly (ACT is closer to PSUM)
                av_sb = avsbp.tile([128, 512], F32, tag="av_sb")
                den_sb = avsbp.tile([128, 512], F32, tag="den_sb")
                nc.scalar.activation(av_sb[:], av[:], COPY)
                nc.scalar.activation(den_sb[:], dn[:], COPY)
                deferred.append((hp, qh, av_sb, den_sb))

        while deferred:
            emit_div(*deferred.pop(0))


def _phase_d(nc, tc, tokT, wproj_sb, bproj_bc, out_e):
    """out[t, c] = tokT.T @ Wproj + bproj."""
    with (
        tc.tile_pool(name="pd_psum", bufs=4, space="PSUM") as pdp,
        tc.tile_pool(name="pd_sbuf", bufs=4) as pds,
    ):
        for tcn in range(8):
            pj = pdp.tile([128, C], F32, tag="pd")
            for cc in range(6):
                for c0, c1 in ((0, 512), (512, C)):
                    nc.tensor.matmul(
                        pj[:, c0:c1],
                        tokT[:, cc, tcn * 128 : (tcn + 1) * 128],
                        wproj_sb[:, cc, c0:c1],
                        start=(cc == 0),
                        stop=(cc == 5),
                    )
            ot = pds.tile([128, C], F32, tag="ot")
            nc.vector.tensor_tensor(
                out=ot[:], in0=pj[:], in1=bproj_bc[:], op=ADD
            )
            nc.sync.dma_start(out_e[tcn * 128 : (tcn + 1) * 128, :], ot[:])


_CACHE = {}


def _get_graph():
    if "nc" not in _CACHE:
        _CACHE["nc"] = build_graph()
    return _CACHE["nc"]


def make_in_maps(x, W_qkv, W_proj, b_proj):
    x = np.asarray(x, dtype=np.float32)
    W_qkv = np.ascontiguousarray(np.asarray(W_qkv, dtype=np.float16))
    W_proj = np.ascontiguousarray(np.asarray(W_proj, dtype=np.float32))
    b_proj = np.asarray(b_proj, dtype=np.float32).reshape(1, C)
    in_maps = []
    for c in range(NCORES):
        bb, r0 = c // 4, (c % 4) * NT
        idx = np.r_[r0 : r0 + NT, 0:r0, r0 + NT : N]
        xT = np.ascontiguousarray(x[bb][idx].T.astype(np.float16))
        in_maps.append(
            {
                "xT": xT,
                "Wqkv": W_qkv,
                "Wproj": W_proj,
                "bproj": b_proj,
            }
        )
    return in_maps


def run(x, W_qkv, W_proj, b_proj, trace=False):
    nc = _get_graph()
    in_maps = make_in_maps(x, W_qkv, W_proj, b_proj)
    res = run_bass_kernel_spmd(
        nc, in_maps, core_ids=list(range(NCORES)), trace=trace
    )
    out = np.zeros((B, N, C), dtype=np.float32)
    for c in range(NCORES):
        bb, r0 = c // 4, (c % 4) * NT
        out[bb, r0 : r0 + NT, :] = res.results[c]["out"]
    return out, res


def kernel(x, W_qkv, W_proj, b_proj):
    out, _ = run(x, W_qkv, W_proj, b_proj, trace=False)
    return out


# revision 19
# speedup vs baseline: 1.3446x; 1.0008x over previous
"""Trainium2 Bass kernel: 12-head self-attention block (B=2, N=4096, C=768).

Sharding: token-parallel over the 8192 (batch, token) rows. Core c (0..7)
handles batch c//4, query rows [(c%4)*1024, (c%4+1)*1024). Every core
redundantly computes K/V for its WHOLE batch - zero collectives. The host
rotates each core's token order so its own 1024 query tokens come first
(attention is permutation-invariant over keys).

v3 (vs v2 629us): PE pair-concurrency + balanced ACT/DVE/GpSimd duties.
  - phase A: x streamed in 512-token half-quarters (double-buffered), Wqkv
    DMA'd in q/k/v column thirds (q first) so the PE ramps immediately.
  - phase C per (head-pair, query-half, key-chunk): three concurrent MM
    pairs - scores (row groups 0/64), AV (col groups 0/64), denominator
    (ones lhsT, M=64 -> the denominator lands replicated across 64
    partitions = broadcast for free). exp: ACT does head A batched
    [128,1024] per 2 chunks; DVE does head B via Schraudolph int16
    bit-trick. Division: reciprocal_approx_fast + one gpsimd multiply per
    pass, deferred off the critical path. No partition-shift DMA: head B
    lands on partitions 64-127 natively.
  - PSUM: scA 2x2 + scB 2x1 + av 1 + den 1 = 8 banks.
  - Wproj/bias prefetched before phase C.
"""

import sys

import numpy as np

try:
    import concourse  # noqa: F401
except ImportError:  # pragma: no cover
    sys.path.insert(0, "/opt/trn_rl_repo")

import concourse.bass as bass  # noqa: F401
import concourse.mybir as mybir
import concourse.tile as tile
from concourse import bacc
from concourse.bass_utils import run_bass_kernel_spmd

B, N, C = 2, 4096, 768
H, D = 12, 64
NT = 1024  # query tokens per core
SCALE = float(D) ** -0.5
NCORES = 8
KC = N // 128  # 32 key chunks per batch

F32 = mybir.dt.float32
F32R = mybir.dt.float32r
BF16 = mybir.dt.bfloat16
FP16 = mybir.dt.float16
I16 = mybir.dt.int16
EXP = mybir.ActivationFunctionType.Exp
COPY = mybir.ActivationFunctionType.Copy
MUL = mybir.AluOpType.mult
ADD = mybir.AluOpType.add

# Schraudolph exp in bf16 bits: e_bits = int16(A_EXP * s + B_EXP)
A_EXP = 1.4426950408889634 * 128.0
B_EXP = 16256.0 - 6.0


def build_graph():
    nc = bacc.Bacc(
        "TRN2", target_bir_lowering=False, debug=False, num_devices=NCORES
    )

    xT_e = nc.declare_dram_parameter("xT", [C, N], FP16, isOutput=False)
    wqkv_e = nc.declare_dram_parameter("Wqkv", [C, 3 * C], FP16, isOutput=False)
    wproj_e = nc.declare_dram_parameter("Wproj", [C, C], F32R, isOutput=False)
    bproj_e = nc.declare_dram_parameter("bproj", [1, C], F32, isOutput=False)
    out_e = nc.declare_dram_parameter("out", [NT, C], F32, isOutput=True)

    with tile.TileContext(nc) as tc:
        with tc.tile_pool(name="persist", bufs=1) as persist:
            kT_sb = persist.tile([128, 6, N], FP16, tag="kT")  # 48K/part
            vt_sb = persist.tile([128, KC, H, D], BF16, tag="vt")  # 48K
            qT_sb = persist.tile([128, 6, NT], FP16, tag="qT")  # 12K
            ones64 = persist.tile([128, 64], BF16, tag="ones64")
            nc.vector.memset(ones64[:], 1.0)

            _phase_a(nc, tc, xT_e, wqkv_e, kT_sb, vt_sb, qT_sb)

            with (
                tc.tile_pool(name="tokp", bufs=1) as tokp,
                tc.tile_pool(name="pd_w", bufs=1) as pdw,
            ):
                tokT = tokp.tile([128, 6, NT], F32R, tag="tokT")  # 24K
                # prefetch Wproj/bias during phase C
                wproj_sb = pdw.tile([128, 6, C], F32R, tag="wproj")
                bproj_sb = pdw.tile([1, C], F32, tag="bproj")
                bproj_bc = pdw.tile([128, C], F32, tag="bproj_bc")
                nc.sync.dma_start(bproj_sb[:], bproj_e[:])
                nc.gpsimd.partition_broadcast(bproj_bc[:], bproj_sb[:])
                for cc in range(6):
                    nc.sync.dma_start(
                        wproj_sb[:, cc, :],
                        wproj_e[cc * 128 : (cc + 1) * 128, :],
                    )
                _phase_c(nc, tc, kT_sb, vt_sb, qT_sb, tokT, ones64)
                _phase_d(nc, tc, tokT, wproj_sb, bproj_bc, out_e)
    nc.finalize()
    return nc


def _phase_a(nc, tc, xT_e, wqkv_e, kT_sb, vt_sb, qT_sb):
    """qkv projection in fp16 (FWL-eligible weight loads), 1024-token
    quarters, double-buffered x, fat DMA slabs (>=2KB per partition)."""
    with (
        tc.tile_pool(name="pa_w", bufs=1) as paw,
        tc.tile_pool(name="pa_x", bufs=2) as pax,
        tc.tile_pool(name="pa_psk", bufs=2, space="PSUM") as papk,
        tc.tile_pool(name="pa_psv", bufs=2, space="PSUM") as papv,
    ):
        wqkv_sb = paw.tile([128, 6, 3 * C], FP16, tag="wqkv")  # 27.6K
        for kc in range(6):
            nc.sync.dma_start(
                wqkv_sb[:, kc, :], wqkv_e[kc * 128 : (kc + 1) * 128, :]
            )

        ncopy = [0]

        def copy_out(dst, src_ap):
            ncopy[0] += 1
            if ncopy[0] & 1:
                nc.vector.tensor_copy(dst, src_ap)
            else:
                nc.scalar.activation(dst, src_ap, COPY)

        for tq in range(4):  # 1024-token quarters
            t0 = tq * NT
            xq = pax.tile([128, 6, NT], FP16, tag="xq")  # 12K x2
            for kc in range(6):
                nc.sync.dma_start(
                    xq[:, kc, :], xT_e[kc * 128 : (kc + 1) * 128, t0 : t0 + NT]
                )

            # q (quarter 0 only, pre-scaled by SCALE) + k columns
            ccs = range(12) if tq == 0 else range(6, 12)
            for cc in ccs:
                pj = papk.tile([128, NT], F32, tag="pj")
                for kc in range(6):
                    for th in range(2):
                        nc.tensor.matmul(
                            pj[:, th * 512 : (th + 1) * 512],
                            wqkv_sb[:, kc, cc * 128 : (cc + 1) * 128],
                            xq[:, kc, th * 512 : (th + 1) * 512],
                            start=(kc == 0),
                            stop=(kc == 5),
                        )
                if cc < 6:
                    nc.scalar.activation(
                        qT_sb[:, cc, t0 : t0 + NT], pj[:], COPY, scale=SCALE
                    )
                else:
                    copy_out(kT_sb[:, cc - 6, t0 : t0 + NT], pj[:])

            # v (token-major), 8 chunks of 128 tokens
            for tcn in range(8):
                pj = papv.tile([128, C], F32, tag="pjv")
                for kc in range(6):
                    for c0, c1 in ((0, 512), (512, C)):
                        nc.tensor.matmul(
                            pj[:, c0:c1],
                            xq[:, kc, tcn * 128 : (tcn + 1) * 128],
                            wqkv_sb[:, kc, 2 * C + c0 : 2 * C + c1],
                            start=(kc == 0),
                            stop=(kc == 5),
                        )
                copy_out(
                    vt_sb[:, tq * 8 + tcn, :, :],
                    pj[:].rearrange("p (h d) -> p h d", d=D),
                )


def _phase_c(nc, tc, kT_sb, vt_sb, qT_sb, tokT, ones64):
    """Attention. Two query-half passes per head pair. Per key chunk kc:
    scores pair (row groups 0/64), AV pair + denominator pair (col groups
    0/64; ones lhsT M=64 replicates the denominator = free broadcast).
    exp: ACT head A [128,1024]/2kc, DVE head B Schraudolph [128,512]/kc."""
    with (
        tc.tile_pool(name="scA", bufs=2, space="PSUM") as scpA,
        tc.tile_pool(name="scB", bufs=2, space="PSUM") as scpB,
        tc.tile_pool(name="avp", bufs=1, space="PSUM") as avp,
        tc.tile_pool(name="dnp", bufs=1, space="PSUM") as dnp,
        tc.tile_pool(name="e0p", bufs=3) as e0p,
        tc.tile_pool(name="e1p", bufs=6) as e1p,
        tc.tile_pool(name="avsb", bufs=2) as avsbp,
        tc.tile_pool(name="recp", bufs=2) as recp,
    ):
        deferred = []  # (hp, qh, av_sb, den_sb)

        def emit_div(hp, qh, av_sb, den_sb):
            qsl = slice(qh * 512, (qh + 1) * 512)
            rec = recp.tile([128, 512], F32, tag="rec")
            nc.vector.reciprocal_approx_fast(rec[:], den_sb[:])
            nc.gpsimd.tensor_tensor(
                out=tokT[:, hp, qsl], in0=av_sb[:], in1=rec[:], op=MUL
            )

        for hp in range(6):
            for qh in range(2):
                qsl = slice(qh * 512, (qh + 1) * 512)
                av = avp.tile([128, 512], F32, tag="av", name=f"av_{hp}_{qh}")
                dn = dnp.tile([128, 512], F32, tag="dn", name=f"dn_{hp}_{qh}")

                def do_avdn(kc, e0s, e1s, av=av, dn=dn, hp=hp):
                    st, sp = kc == 0, kc == KC - 1
                    nc.tensor.matmul(
                        av[0:64, :], vt_sb[:, kc, 2 * hp, :], e0s,
                        start=st, stop=sp, tile_position=(0, 0),
                    )
                    nc.tensor.matmul(
                        av[64:128, :], vt_sb[:, kc, 2 * hp + 1, :], e1s,
                        start=st, stop=sp, tile_position=(0, 64),
                    )
                    nc.tensor.matmul(
                        dn[0:64, :], ones64[:], e0s,
                        start=st, stop=sp, tile_position=(0, 0),
                    )
                    nc.tensor.matmul(
                        dn[64:128, :], ones64[:], e1s,
                        start=st, stop=sp, tile_position=(0, 64),
                    )

                pend = []
                scA = None
                for kc in range(KC):
                    if kc == 8 and deferred:
                        emit_div(*deferred.pop(0))
                    if pend and kc >= 2:
                        do_avdn(*pend.pop(0))
                    ksl = slice(kc * 128, (kc + 1) * 128)
                    half = (kc % 2) * 512
                    if kc % 2 == 0:
                        scA = scpA.tile([128, 1024], F32, tag="scA")
                    scB = scpB.tile([128, 512], F32, tag="scB")
                    nc.tensor.matmul(
                        scA[:, half : half + 512],
                        kT_sb[0:64, hp, ksl],
                        qT_sb[0:64, hp, qsl],
                        start=True,
                        stop=True,
                    )
                    nc.tensor.matmul(
                        scB[:],
                        kT_sb[64:128, hp, ksl],
                        qT_sb[64:128, hp, qsl],
                        start=True,
                        stop=True,
                    )
                    e1i = e1p.tile([128, 512], I16, tag="e1")
                    nc.vector.tensor_scalar(
                        e1i[:], scB[:], A_EXP, B_EXP, MUL, ADD
                    )
                    if kc % 2 == 1:
                        e0 = e0p.tile([128, 1024], BF16, tag="e0")
                        nc.scalar.activation(e0[:], scA[:], EXP, scale=1.0)
                        pend.append((kc - 1, e0[:, 0:512], e1prev[:].bitcast(BF16)))
                        pend.append((kc, e0[:, 512:1024], e1i[:].bitcast(BF16)))
                    e1prev = e1i
                while pend:
                    do_avdn(*pend.pop(0))

                # evacuate av/den PSUM promptly - one copy per engine so
                # neither queue delays the next pass's first AV matmul
                av_sb = avsbp.tile([128, 512], F32, tag="av_sb")
                den_sb = avsbp.tile([128, 512], F32, tag="den_sb")
                nc.scalar.activation(av_sb[:], av[:], COPY)
                nc.vector.tensor_copy(den_sb[:], dn[:])
                deferred.append((hp, qh, av_sb, den_sb))

        while deferred:
            emit_div(*deferred.pop(0))


def _phase_d(nc, tc, tokT, wproj_sb, bproj_bc, out_e):
    """out[t, c] = tokT.T @ Wproj + bproj."""
    with (
        tc.tile_pool(name="pd_psum", bufs=4, space="PSUM") as pdp,
        tc.tile_pool(name="pd_sbuf", bufs=4) as pds,
    ):
        for tcn in range(8):
            pj = pdp.tile([128, C], F32, tag="pd")
            for cc in range(6):
                for c0, c1 in ((0, 512), (512, C)):
                    nc.tensor.matmul(
                        pj[:, c0:c1],
                        tokT[:, cc, tcn * 128 : (tcn + 1) * 128],
                        wproj_sb[:, cc, c0:c1],
                        start=(cc == 0),
                        stop=(cc == 5),
                    )
            ot = pds.tile([128, C], F32, tag="ot")
            nc.vector.tensor_tensor(
                out=ot[:], in0=pj[:], in1=bproj_bc[:], op=ADD
            )
            nc.sync.dma_start(out_e[tcn * 128 : (tcn + 1) * 128, :], ot[:])


_CACHE = {}


def _get_graph():
    if "nc" not in _CACHE:
        _CACHE["nc"] = build_graph()
    return _CACHE["nc"]


def make_in_maps(x, W_qkv, W_proj, b_proj):
    x = np.asarray(x, dtype=np.float32)
    W_qkv = np.ascontiguousarray(np.asarray(W_qkv, dtype=np.float16))
    W_proj = np.ascontiguousarray(np.asarray(W_proj, dtype=np.float32))
    b_proj = np.asarray(b_proj, dtype=np.float32).reshape(1, C)
    in_maps = []
    for c in range(NCORES):
        bb, r0 = c // 4, (c % 4) * NT
        idx = np.r_[r0 : r0 + NT, 0:r0, r0 + NT : N]
        xT = np.ascontiguousarray(x[bb][idx].T.astype(np.float16))
        in_maps.append(
            {
                "xT": xT,
                "Wqkv": W_qkv,
                "Wproj": W_proj,
                "bproj": b_proj,
            }
        )
    return in_maps


def run(x, W_qkv, W_proj, b_proj, trace=False):
    nc = _get_graph()
    in_maps = make_in_maps(x, W_qkv, W_proj, b_proj)
    res = run_bass_kernel_spmd(
        nc, in_maps, core_ids=list(range(NCORES)), trace=trace
    )
    out = np.zeros((B, N, C), dtype=np.float32)
    for c in range(NCORES):
        bb, r0 = c // 4, (c % 4) * NT
        out[bb, r0 : r0 + NT, :] = res.results[c]["out"]
    return out, res


def kernel(x, W_qkv, W_proj, b_proj):
    out, _ = run(x, W_qkv, W_proj, b_proj, trace=False)
    return out
